# revision 22
# baseline (speedup 1.0000x reference)
"""Trainium2 Bass kernel for GQA attention with RoPE (B=2, S=1024, HID=2048,
16 q heads / 4 kv heads, head dim 128, causal).

Sharding: 8 cores = 2 batches x 4 kv-head groups. Core c = b*4 + g handles
batch b and kv head g (query heads 4g..4g+3). Each core computes a partial
output y_part = attn_heads @ wo_shard; the host sums the 4 partials per batch.

v2: all-bf16 datapath. x arrives pre-transposed/pre-chunked from the host
(xT[g][p, k, s]), so the PE transpose stage is gone entirely. Weights arrive
bf16 pre-chunked. Matmuls run bf16 (1 cyc/row), RoPE/copies run bf16 on DVE
(2x mode). Output is written bf16; host upcasts and reduces in fp32.

Per-core dataflow:
  Phase A (per 128-row chunk g, pipelined 1 deep):
    qkv_ps = sum_k xT[g,k].T @ wqkv[k]  (PE, fp32 psum)
    qkv_sb = copy(qkv_ps) bf16          (ACT)
    RoPE on DVE (bf16); v chunk copied to vv
    PE transpose q_rope/k_rope -> persistent qT[d,h,s], kT[d,s]
  Phase B/C (per 256-col macro tile, heads pipelined one deep):
    scoresT[sk,sq] = kT_chunk.T @ qT ; mask add (DVE); expS (ACT, bf16)
    denom_rep = ones.T @ expS ; U^T = v.T @ expS   (PE, accumulated)
    rec = 1/denom (DVE) ; uT = U^T * rec  (DVE, bf16)
    y = sum_h uT_h.T @ wo_h  (PE) -> bf16 SBUF (DVE/ACT alternating) -> DRAM
"""

import sys

import numpy as np

for _p in ("/opt/trn_rl_repo", "/root/.axon_site/_ro/trn_rl_repo"):
    if _p not in sys.path:
        sys.path.append(_p)

from contextlib import ExitStack

import ml_dtypes

import concourse.bass as bass
import concourse.mybir as mybir
from concourse import bacc
from concourse.masks import make_identity
from concourse.tile import TileContext

P = 128           # partitions / head dim / seq chunk
S = 1024          # sequence length
HID = 2048        # model dim
NH = 4            # query heads per core
D = 128           # head dim
TQ = 256          # query macro-tile (matmul moving free dim)
NT = S // TQ      # 4 macro tiles
KC = HID // P     # 16 contraction chunks
NSK = S // P      # 8 key chunks
NG = S // P       # 8 row chunks
F32 = mybir.dt.float32
BF16 = mybir.dt.bfloat16
SCALE = 1.0 / float(np.sqrt(D))
NEG = -30000.0
AL = mybir.AluOpType
AF = mybir.ActivationFunctionType

N_CORES = 8
B = 2
N_KV = 4

BF = ml_dtypes.bfloat16


def build_nc():
    nc = bacc.Bacc("TRN2", target_bir_lowering=False, debug=False)
    # host-prepped layouts (bf16):
    #   xT:   [NG, P, KC, P]  xT[g, p, k, s] = x[g*128+s, k*128+p]
    #   wqkv: [P, KC, 768]    wqkv[p, k, n]  = [wq|wk|wv][k*128+p, n]
    #   wo:   [P, NH, HID]    wo[p, h, n]    = wo[h*128+p, n]
    xT_d = nc.declare_dram_parameter("xT", [NG, P, KC, P], BF16, isOutput=False)
    cos_d = nc.declare_dram_parameter("cos", [S, D], BF16, isOutput=False)
    sin_d = nc.declare_dram_parameter("sin", [S, D], BF16, isOutput=False)
    wqkv_d = nc.declare_dram_parameter("wqkv", [P, KC, NH * D + 2 * D], BF16, isOutput=False)
    wo_d = nc.declare_dram_parameter("wo", [P, NH, HID], BF16, isOutput=False)
    out_d = nc.declare_dram_parameter("out", [S, HID], BF16, isOutput=True)

    with TileContext(nc) as tc, ExitStack() as ctx:
        lp = nc.allow_low_precision(reason="bf16 kernel, validated vs reference")
        lp.__enter__()
        consts = ctx.enter_context(tc.tile_pool(name="consts", bufs=1))
        wpool = ctx.enter_context(tc.tile_pool(name="wpool", bufs=1))
        persist = ctx.enter_context(tc.tile_pool(name="persist", bufs=1))

        # ---- constants ----
        ident_f32 = consts.tile([P, P], F32, tag="ident_f32")
        make_identity(nc, ident_f32)
        ident = consts.tile([P, P], BF16, tag="ident")
        nc.vector.tensor_copy(ident, ident_f32)
        ones_f32 = consts.tile([P, P], F32, tag="ones_f32")
        nc.vector.memset(ones_f32, 1.0)
        ones = consts.tile([P, P], BF16, tag="ones")
        nc.vector.tensor_copy(ones, ones_f32)

        # ---- weights / activations (bf16 SBUF) ----
        xT_sb = wpool.tile([P, NG, KC, P], BF16, tag="xT")
        wqkv_sb = wpool.tile([P, KC, NH * D + 2 * D], BF16, tag="wqkv")
        wo_sb = wpool.tile([P, NH, HID], BF16, tag="wo")
        cos_sb = wpool.tile([P, NG, D], BF16, tag="cos")
        sin_sb = wpool.tile([P, NG, D], BF16, tag="sin")

        # persistent transposed activations
        qT_all = persist.tile([P, NH, S], BF16, tag="qT")   # [d, h, sq]
        kT = persist.tile([P, S], BF16, tag="kT")           # [d, sk]
        vv = persist.tile([P, NSK, D], BF16, tag="vv")      # v natural [sk, d]

        H2 = D // 2

        def bc(small, big):
            """Broadcast [P, X] AP against [P, n, X] AP (stride-0 middle dim)."""
            s3 = small.rearrange("p (o d) -> p o d", o=1)
            a, b = bass.broadcast_tensor_aps(big, s3)
            return b

        def rope_multi(dst, src, g, n, tmp_tag, wk):
            """dst[:, i, :] = src_i*cos + rotate_half(src_i)*sin for n heads.

            src/dst are [P, n, D] APs; cos/sin broadcast across the head dim
            so the whole chunk is 4 DVE ops regardless of n."""
            cos_g = cos_sb[:, g, :]
            sin_g = sin_sb[:, g, :]
            tmp = wk.tile([P, n, D], BF16, tag=tmp_tag)
            nc.vector.scalar_tensor_tensor(
                out=tmp[:, :, 0:H2], in0=src[:, :, H2:D], scalar=-1.0,
                in1=bc(sin_g[:, 0:H2], tmp[:, :, 0:H2]),
                op0=AL.mult, op1=AL.mult,
            )
            nc.vector.tensor_tensor(
                out=tmp[:, :, H2:D], in0=src[:, :, 0:H2],
                in1=bc(sin_g[:, H2:D], tmp[:, :, H2:D]), op=AL.mult
            )
            nc.vector.tensor_tensor(
                out=dst, in0=src, in1=bc(cos_g, dst), op=AL.mult
            )
            nc.vector.tensor_tensor(out=dst, in0=dst, in1=tmp, op=AL.add)

        # ================= fused pipeline =================
        pa = ctx.enter_context(tc.tile_pool(name="pa", bufs=2))
        pb = ctx.enter_context(tc.tile_pool(name="pb", bufs=2))
        ps_mega = ctx.enter_context(tc.tile_pool(name="ps_mega", bufs=3, space="PSUM"))
        ps_du = ctx.enter_context(tc.tile_pool(name="ps_du", bufs=2, space="PSUM"))
        ps_qkv = ctx.enter_context(tc.tile_pool(name="ps_qkv", bufs=1, space="PSUM"))
        ps_tp = ctx.enter_context(tc.tile_pool(name="ps_tp", bufs=1, space="PSUM"))

        # dummy matmuls to lift the PE HAM clock gate to 8/8 while the
        # first weight/x DMAs are in flight
        warm_ps = ps_mega.tile([P, 512], F32, tag="mega", name="warm")
        for _ in range(60):
            nc.tensor.matmul(warm_ps[:, 0:P], ones, ones, start=True, stop=True)
        warm_drain = pa.tile([P, 4], F32, tag="warmdrain", bufs=1)
        nc.vector.tensor_copy(warm_drain, warm_ps[:, 0:4])

        def emit_xdma(g):
            nc.sync.dma_start(out=xT_sb[:, g, :, :], in_=xT_d[g])

        # DMA order: wqkv in 4 chunk-pieces so proj g0's accumulation can
        # start on piece 1 (subtile deps); x chunks and cos/sin interleaved,
        # wo trailing
        nc.sync.dma_start(out=wqkv_sb[:, 0:4, :], in_=wqkv_d[:, 0:4, :])
        emit_xdma(0)
        nc.sync.dma_start(out=wqkv_sb[:, 4:8, :], in_=wqkv_d[:, 4:8, :])
        nc.sync.dma_start(out=wqkv_sb[:, 8:12, :], in_=wqkv_d[:, 8:12, :])
        nc.sync.dma_start(out=wqkv_sb[:, 12:16, :], in_=wqkv_d[:, 12:16, :])
        emit_xdma(1)
        emit_xdma(2)
        emit_xdma(3)
        nc.sync.dma_start(
            out=cos_sb, in_=cos_d[:].rearrange("(c p) d -> p c d", p=P)
        )
        nc.sync.dma_start(
            out=sin_sb, in_=sin_d[:].rearrange("(c p) d -> p c d", p=P)
        )
        wo_next = [0]

        def emit_wo_dma():
            h = wo_next[0]
            if h < NH:
                nc.sync.dma_start(out=wo_sb[:, h, :], in_=wo_d[:, h, :])
                wo_next[0] += 1

        def proj(g):
            """qkv projections for chunk g (PE, accumulating over hid chunks);
            result copied to bf16 SBUF via ACT to free the PSUM bank."""
            qkv_ps = ps_qkv.tile([P, NH * D + 2 * D], F32, tag="qkv")
            q_ps = qkv_ps[:, 0 : NH * D]
            kv_ps = qkv_ps[:, NH * D : NH * D + 2 * D]
            for k in range(KC):
                st, sp = (k == 0), (k == KC - 1)
                xk = xT_sb[:, g, k, :]
                nc.tensor.matmul(q_ps, xk, wqkv_sb[:, k, 0 : NH * D], start=st, stop=sp)
                nc.tensor.matmul(
                    kv_ps, xk, wqkv_sb[:, k, NH * D : NH * D + 2 * D],
                    start=st, stop=sp,
                )
            qkv_sb = pa.tile([P, NH * D + 2 * D], BF16, tag="qkvsb", bufs=3)
            nc.scalar.activation(
                out=qkv_sb[:, 0 : NH * D], in_=q_ps, func=AF.Copy
            )
            nc.vector.tensor_copy(qkv_sb[:, NH * D : NH * D + 2 * D], kv_ps)
            return qkv_sb

        def rope_stage(g, qkv_sb):
            """RoPE on q heads + k (DVE bf16, head-batched), v copy-out."""
            q3 = qkv_sb[:, 0 : NH * D].rearrange("p (h d) -> p h d", h=NH)
            kv_sb = qkv_sb[:, NH * D : NH * D + 2 * D]
            q_rope = pa.tile([P, NH, D], BF16, tag="qrope")
            rope_multi(q_rope[:], q3, g, NH, "tmq", pa)
            k_rope = pa.tile([P, D], BF16, tag="krope")
            rope_multi(
                k_rope.rearrange("p (o d) -> p o d", o=1),
                kv_sb[:, 0:D].rearrange("p (o d) -> p o d", o=1),
                g, 1, "tmk", pa,
            )
            nc.vector.tensor_copy(vv[:, g, :], kv_sb[:, D : 2 * D])
            return q_rope, k_rope

        def rope_transpose(g, q_rope, k_rope):
            """Transpose RoPE'd q/k into persistent qT_all / kT (bf16 psum)."""
            tp_ps = ps_tp.tile([P, NH + 1, P], BF16, tag="tp")
            for h in range(NH):
                nc.tensor.transpose(tp_ps[:, h, :], q_rope[:, h, :], ident)
            nc.tensor.transpose(tp_ps[:, NH, :], k_rope, ident)
            nc.vector.tensor_copy(
                qT_all[:, :, g * P : (g + 1) * P], tp_ps[:, 0:NH, :]
            )
            nc.vector.tensor_copy(kT[:, g * P : (g + 1) * P], tp_ps[:, NH, :])

        # 2-deep software pipeline over chunks: proj runs two chunks ahead
        # of rope/ropeT so the DVE rope chain never stalls the PE
        pend = [None] * NG

        def emit_phase_a(g):
            if g < NG:
                if g + 4 < NG:
                    emit_xdma(g + 4)
                if g >= 2:
                    emit_wo_dma()
                    emit_wo_dma()
                sc = nc.named_scope(f"proj_{g}"); sc.__enter__()
                pend[g] = proj(g)
                sc.__exit__(None, None, None)
            if g >= 2:
                gg = g - 2
                sc = nc.named_scope(f"rope_{gg}"); sc.__enter__()
                qr, kr = rope_stage(gg, pend[gg])
                sc.__exit__(None, None, None)
                sc = nc.named_scope(f"ropeT_{gg}"); sc.__enter__()
                rope_transpose(gg, qr, kr)
                sc.__exit__(None, None, None)
                pend[gg] = None

        def scores_head(t, h):
            """scoresT + exp for head h of macro tile t -> expst tile (bf16).

            Chunk pairs share one full PSUM bank; exp is one ACT op per pair.
            The causal mask is applied post-exp by zeroing the invalid
            triangle of the diagonal pair on gpsimd (exp(-inf) == 0)."""
            qT_h = qT_all[:, h, t * TQ : (t + 1) * TQ]
            expst = pb.tile([P, NSK, TQ], BF16, tag="expst", bufs=3)
            expst_flat = expst.rearrange("p c f -> p (c f)")
            for pi in range(t + 1):
                s_ps = ps_mega.tile([P, 2 * TQ], F32, tag="mega", name="s")
                for half in range(2):
                    ik = 2 * pi + half
                    nc.tensor.matmul(
                        s_ps[:, half * TQ : (half + 1) * TQ],
                        kT[:, ik * P : (ik + 1) * P], qT_h,
                        start=True, stop=True,
                    )
                nc.scalar.activation(
                    out=expst_flat[:, pi * 2 * TQ : (pi + 1) * 2 * TQ],
                    in_=s_ps, func=AF.Exp, scale=SCALE,
                )
            # zero exp values where key > query: chunk c of the diagonal
            # pair keeps entries with j - 128*c - p >= 0
            nc.gpsimd.affine_select(
                out=expst[:, 2 * t : 2 * t + 2, :],
                in_=expst[:, 2 * t : 2 * t + 2, :],
                compare_op=AL.is_ge, fill=0.0,
                base=0, pattern=[[-P, 2], [1, TQ]], channel_multiplier=-1,
            )
            return expst

        def dnpv_head(t, h, expst, uT_t):
            """denominator + PV matmuls, then normalize into uT_t (DVE).

            rec = exp(-ln(den)) on ACT: Ln and Exp share one table set with
            the softmax Exp, so no ACT table reloads; DVE reciprocal would
            cost 1.75us per call."""
            nsk = 2 * (t + 1)
            du_ps = ps_du.tile([P, 2 * TQ], F32, tag="du")
            den_ps = du_ps[:, 0:TQ]
            u_ps = du_ps[:, TQ : 2 * TQ]
            for ik in range(nsk):
                nc.tensor.matmul(
                    den_ps, ones, expst[:, ik, :],
                    start=(ik == 0), stop=(ik == nsk - 1),
                )
            rec = pb.tile([P, TQ], F32, tag="rec", bufs=2)
            nc.vector.reciprocal(rec, den_ps)
            for ik in range(nsk):
                nc.tensor.matmul(
                    u_ps, vv[:, ik, :], expst[:, ik, :],
                    start=(ik == 0), stop=(ik == nsk - 1),
                )
            nc.vector.tensor_tensor(
                out=uT_t[:, h, :], in0=u_ps, in1=rec, op=AL.mult
            )

        y_rr = [0]

        def wo_stage(t, uT_t):
            for sub in range(2):
                g = 2 * t + sub
                for n in range(HID // 512):
                    y_ps = ps_mega.tile([P, 512], F32, tag="mega", name="y")
                    for h in range(NH):
                        nc.tensor.matmul(
                            y_ps,
                            uT_t[:, h, sub * P : (sub + 1) * P],
                            wo_sb[:, h, n * 512 : (n + 1) * 512],
                            start=(h == 0), stop=(h == NH - 1),
                        )
                    y_sb = pb.tile([P, 512], BF16, tag="ysb", bufs=3)
                    r = y_rr[0] % 2
                    y_rr[0] += 1
                    if r == 0:
                        nc.vector.tensor_copy(y_sb, y_ps)
                    else:
                        nc.scalar.activation(out=y_sb, in_=y_ps, func=AF.Copy)
                    nc.gpsimd.dma_start(
                        out=out_d[g * P : (g + 1) * P, n * 512 : (n + 1) * 512],
                        in_=y_sb,
                    )

        # attention head-steps, scores pipelined two deep ahead of dnpv so
        # the ACT exp chain never gates the PE; wo lags dnpv completion of
        # each tile; consumed interleaved with phase-A iterations
        LAG = 2
        steps = [(t, h) for t in range(NT) for h in range(NH)]
        uts = {}
        att_i = [0]

        def emit_attention_step():
            i = att_i[0]
            if i >= len(steps) + LAG + 1:
                return False
            if i < len(steps):
                t, h = steps[i]
                if h == 0:
                    uts[t] = pb.tile([P, NH, TQ], BF16, tag="uT", name=f"uT{t}")
                sc = nc.named_scope(f"sc_{t}_{h}"); sc.__enter__()
                uts[(t, h)] = scores_head(t, h)
                sc.__exit__(None, None, None)
            if LAG <= i < len(steps) + LAG:
                t, h = steps[i - LAG]
                sc = nc.named_scope(f"dnpv_{t}_{h}"); sc.__enter__()
                dnpv_head(t, h, uts.pop((t, h)), uts[t])
                sc.__exit__(None, None, None)
            if i >= LAG + 1 and (i - LAG - 1) % NH == NH - 1:
                t = steps[i - LAG - 1][0]
                sc = nc.named_scope(f"wo_{t}"); sc.__enter__()
                wo_stage(t, uts.pop(t))
                sc.__exit__(None, None, None)
            att_i[0] += 1
            return True

        # drive: phase-A iteration g, then any attention steps whose
        # inputs (kT/v/qT up to chunk 2t+1) are complete after ropeT_{g-2}
        for g in range(NG + 2):
            emit_phase_a(g)
            done_g = g - 2  # ropeT for this chunk just emitted
            while att_i[0] < len(steps) + LAG + 1:
                i = att_i[0]
                if i < len(steps):
                    t, _h = steps[i]
                    if 2 * t + 1 > done_g:
                        break
                emit_attention_step()
        emit_wo_dma()
        emit_wo_dma()
        emit_wo_dma()
        emit_wo_dma()
        while emit_attention_step():
            pass
        lp.__exit__(None, None, None)

    nc.compile()
    return nc


def _prep_xt(xb):
    """x[b] [S, HID] fp32 -> [NG, P, KC, P] bf16 with
    xT[g, p, k, s] = x[g*128+s, k*128+p]."""
    xt = np.ascontiguousarray(xb.T)                     # [HID, S]
    xt = xt.reshape(KC, P, NG, P).transpose(2, 1, 0, 3)  # [g, p, k, s]
    return np.ascontiguousarray(xt.astype(BF))


def _prep_wqkv(wq_g, wk_g, wv_g):
    w = np.concatenate([wq_g, wk_g, wv_g], axis=1)       # [HID, 768]
    w = w.reshape(KC, P, NH * D + 2 * D).transpose(1, 0, 2)
    return np.ascontiguousarray(w.astype(BF))


def _prep_wo(wo_g):
    w = wo_g.reshape(NH, P, HID).transpose(1, 0, 2)      # [p, h, n]
    return np.ascontiguousarray(w.astype(BF))


def shard_inputs(x, cos, sin, wq, wk, wv, wo):
    """Build per-core input maps: core = b*4 + g."""
    cos_b = np.ascontiguousarray(cos.astype(BF))
    sin_b = np.ascontiguousarray(sin.astype(BF))
    xts = [_prep_xt(x[b]) for b in range(B)]
    wqkvs = [
        _prep_wqkv(
            wq[:, g * NH * D : (g + 1) * NH * D],
            wk[:, g * D : (g + 1) * D],
            wv[:, g * D : (g + 1) * D],
        )
        for g in range(N_KV)
    ]
    wos = [_prep_wo(wo[g * NH * D : (g + 1) * NH * D, :]) for g in range(N_KV)]
    in_maps = []
    for c in range(N_CORES):
        b, g = divmod(c, N_KV)
        in_maps.append(
            {
                "xT": xts[b],
                "cos": cos_b,
                "sin": sin_b,
                "wqkv": wqkvs[g],
                "wo": wos[g],
            }
        )
    return in_maps


_NC_CACHE = {}


def get_nc():
    if "nc" not in _NC_CACHE:
        _NC_CACHE["nc"] = build_nc()
    return _NC_CACHE["nc"]


def kernel(x, cos, sin, wq, wk, wv, wo, _trace=False):
    from concourse.bass_utils import run_bass_kernel_spmd

    x = np.asarray(x, dtype=np.float32)
    cos = np.asarray(cos, dtype=np.float32)
    sin = np.asarray(sin, dtype=np.float32)
    wq = np.asarray(wq, dtype=np.float32)
    wk = np.asarray(wk, dtype=np.float32)
    wv = np.asarray(wv, dtype=np.float32)
    wo = np.asarray(wo, dtype=np.float32)

    nc = get_nc()
    in_maps = shard_inputs(x, cos, sin, wq, wk, wv, wo)
    res = run_bass_kernel_spmd(nc, in_maps, list(range(N_CORES)), trace=_trace)
    parts = [np.asarray(res.results[c]["out"], dtype=np.float32) for c in range(N_CORES)]
    y = np.stack(
        [sum(parts[b * N_KV + g] for g in range(N_KV)) for b in range(B)], axis=0
    )
    if _trace:
        kernel.last_result = res
    return y
